# revision 1
# baseline (speedup 1.0000x reference)
"""Trainium2 Bass kernel for DenseCapsule dynamic routing (3 iterations).

Problem: x[128,2048,8] f32, weight[16,2048,16,8] f32 -> out[128,16,16] f32.
  x_hat = einsum('oide,bie->boid', W, x); 3 routing iterations
  (softmax over o, c-weighted i-sum, squash, agreement update).

Strategy (8 NeuronCores, shard in_num_caps I=2048 -> 256 per core), v2:
  x_hat never materialized; everything factors through W:
    u = v.W (PE), l = sum_e x*u (DVE), softmax (ACT/DVE), xc = c*x (DVE),
    s = xc @ W (PE, col-tiled per-o with D padded to 32 so the PSUM comes out
    directly in the transposed [(g,dd32),(h,b)] layout that the next
    iteration's u-matmul stationary wants -- no PE transposes per iteration).
  Squash uses only Ln/Exp activations (one ACT table set, shared with the
  softmax exp): scale = exp(0.5*ln(n2) - ln(1+n2)) = sqrt(n2)/(1+n2).
  Cross-core: AllReduce after iter1 (one, [B,256] bf16) and two pipelined
  half-ARs after iter2 (o 0..7 while o 8..15 still computing). The final
  iteration's partial s is DMA'd out per-core; the host does the gather-sum
  + final squash in f64.
  PE HAM heartbeats (tiny matmuls dependency-chained to the DVE tape) keep
  the tensor engine at 2.4 GHz through the DVE-heavy phases.

Layout conventions per core (SBUF partition dim first):
  i_local = ih*128 + il  (ih in {0,1}, il = partition 0..127)
  o = 4*h + g            (g in 0..3, h in 0..3)
  vT / s-transposed layout: [(g, dd32) = 128 partitions, (h, b) free]
"""

import sys

for _p in ("/opt/trn_rl_repo", "/root/.axon_site/_ro/trn_rl_repo"):
    if _p not in sys.path:
        sys.path.insert(0, _p)

import numpy as np
import ml_dtypes

import concourse.bass as bass
import concourse.bacc as bacc
import concourse.mybir as mybir
import concourse.tile as tile
from concourse.bass_utils import run_bass_kernel_spmd

F32 = mybir.dt.float32
BF16 = mybir.dt.bfloat16
NPBF16 = ml_dtypes.bfloat16
AF = mybir.ActivationFunctionType
ALU = mybir.AluOpType

N_CORES = 8
B = 128          # batch
I_FULL = 2048    # in caps
IC = 256         # in caps per core
IL = 128         # partition dim of i
IH = IC // IL    # 2
E = 8            # in cap dim
O = 16           # out caps
D = 16           # out cap dim
DD = 32          # padded out cap dim (for 32-aligned col tiling)

_CACHE = {}


def build():
    nc = bacc.Bacc("TRN2", target_bir_lowering=False, debug=False,
                   enable_asserts=True, num_devices=N_CORES)

    xbf_d = nc.dram_tensor("xbf", [IL, IH * E * B], BF16,
                           kind="ExternalInput").ap()
    wdt_d = nc.dram_tensor("wdt", [128, 4 * IH * E * IL], BF16,
                           kind="ExternalInput").ap()
    wst_d = nc.dram_tensor("wst", [IL, IH * E * O * D], BF16,
                           kind="ExternalInput").ap()
    sel4_d = nc.dram_tensor("sel4", [128, 4], BF16, kind="ExternalInput").ap()
    selT_d = nc.dram_tensor("selT", [4, 128], BF16, kind="ExternalInput").ap()
    ident_d = nc.dram_tensor("ident", [128, 128], BF16,
                             kind="ExternalInput").ap()
    sp_out = nc.dram_tensor("sp", [128, 4 * B], BF16,
                        kind="ExternalOutput").ap()

    ar1_in = nc.dram_tensor("ar1_in", [B, O * D], BF16)
    ar1_out = nc.dram_tensor("ar1_out", [B, O * D], BF16, addr_space="Shared")
    ar2a_in = nc.dram_tensor("ar2a_in", [128, 2 * B], BF16)
    ar2a_out = nc.dram_tensor("ar2a_out", [128, 2 * B], BF16,
                              addr_space="Shared")
    ar2b_in = nc.dram_tensor("ar2b_in", [128, 2 * B], BF16)
    ar2b_out = nc.dram_tensor("ar2b_out", [128, 2 * B], BF16,
                              addr_space="Shared")

    rg = [list(range(N_CORES))]

    with tile.TileContext(nc) as tc:
        with (
            tc.tile_pool(name="const", bufs=1) as cpool,
            tc.tile_pool(name="work", bufs=2) as wpool,
            tc.tile_pool(name="psu", bufs=2, space="PSUM") as psu,
            tc.tile_pool(name="pss", bufs=1, space="PSUM") as pss,
            tc.tile_pool(name="psq", bufs=1, space="PSUM") as psq,
        ):
            # ---- load inputs ----
            xbf = cpool.tile([IL, IH * E * B], BF16)
            nc.sync.dma_start(out=xbf[:, :], in_=xbf_d)
            wdt = cpool.tile([128, 4 * IH * E * IL], BF16)
            wst = cpool.tile([IL, IH * E * O * D], BF16)
            nc.sync.dma_start(out=wst[:, :], in_=wst_d)
            sel4 = cpool.tile([128, 4], BF16)
            nc.sync.dma_start(out=sel4[:, :], in_=sel4_d)
            selT = cpool.tile([4, 128], BF16)
            nc.sync.dma_start(out=selT[:, :], in_=selT_d)
            ident = cpool.tile([128, 128], BF16)
            nc.sync.dma_start(out=ident[:, :], in_=ident_d)

            # persistent state
            l_buf = cpool.tile([IL, O * IH * B], BF16)
            delta_buf = cpool.tile([IL, O * IH * B], BF16)
            exp_buf = cpool.tile([IL, O * IH * B], BF16)
            xp = cpool.tile([IL, IH * E * B], BF16)
            vT1 = cpool.tile([128, 4 * B], BF16)
            vT2 = cpool.tile([128, 4 * B], BF16)
            sTa = cpool.tile([128, 2 * B], BF16)
            sTb = cpool.tile([128, 2 * B], BF16)
            c1sb = cpool.tile([B, O * D], BF16)
            c2a = cpool.tile([128, 2 * B], BF16)
            c2b = cpool.tile([128, 2 * B], BF16)
            s1full = cpool.tile([B, O * D], BF16)
            v1pad = cpool.tile([B, O * DD], BF16)
            sp_sb = cpool.tile([128, 4 * B], BF16)
            tblw = cpool.tile([128, 16], F32)

            wst_v = wst[:, :].rearrange("p (t o d) -> p t o d", t=IH * E, o=O)

            I32 = mybir.dt.int32
            mg128 = cpool.tile([128, 2 * B], I32)
            nc.vector.memset(mg128[:, :], 0x5F3759DF)

            def emit_scale(n2v, scl_out, P, N, tag):
                """scl_out (bf16) = sqrt(n2)/(1+n2) via DVE only (quake
                rsqrt + fast reciprocal). Shared bufs=1 tags WAR-chain
                successive squash halves so the scheduler cannot hoist a
                later (AR-gated) half ahead of this one's tail."""
                sh = wpool.tile([P, N], I32, tag="sc_sh", bufs=1)
                nc.vector.tensor_scalar(sh[:, :], n2v.bitcast(I32), 1, None,
                                        op0=ALU.logical_shift_right)
                y0i = wpool.tile([P, N], I32, tag="sc_y0", bufs=1)
                nc.vector.tensor_tensor(y0i[:, :], mg128[0:P, 0:N], sh[:, :],
                                        op=ALU.subtract)
                y0 = y0i[:, :].bitcast(F32)
                t = wpool.tile([P, N], F32, tag="sc_t", bufs=1)
                nc.vector.tensor_tensor(t[:, :], y0, y0, op=ALU.mult)
                h = wpool.tile([P, N], F32, tag="sc_h", bufs=1)
                nc.vector.scalar_tensor_tensor(h[:, :], t[:, :], -0.5,
                                               n2v, op0=ALU.mult,
                                               op1=ALU.mult)
                rsq = wpool.tile([P, N], F32, tag="sc_rq", bufs=1)
                nc.vector.scalar_tensor_tensor(rsq[:, :], h[:, :], 1.5, y0,
                                               op0=ALU.add, op1=ALU.mult)
                w = wpool.tile([P, N], F32, tag="sc_w", bufs=1)
                nc.vector.tensor_scalar_add(w[:, :], n2v, 1.0)
                rw = wpool.tile([P, N], F32, tag="sc_rw", bufs=1)
                nc.vector.reciprocal_approx_fast(rw[:, :], w[:, :])
                nr = wpool.tile([P, N], F32, tag="sc_nr", bufs=1)
                nc.vector.tensor_tensor(nr[:, :], n2v, rsq[:, :], op=ALU.mult)
                nc.vector.tensor_tensor(scl_out, nr[:, :], rw[:, :],
                                        op=ALU.mult)


            def hb(dep_ap, ps_tile):
                """HAM heartbeat: tiny matmul gated on a DVE-produced tile."""
                nc.tensor.matmul(ps_tile[0:4, 2 * B:2 * B + 64], sel4[:, :],
                                 dep_ap, start=True, stop=True)

            # ---- s1: uniform c -> s1 = (1/16) x @ W  (out [b, (o,dd)]) ----
            _sid, _ = nc.enter_named_scope("s1", False)
            s1_ps = psu.tile([B, O * D], F32, tag="u")
            for t in range(IH * E):
                lhsT = xbf[:, t * B:(t + 1) * B]
                rhs = wst_v[:, t]  # [IL, O*DD]
                nc.tensor.matmul(s1_ps[:, :], lhsT, rhs,
                                 start=(t == 0), stop=(t == IH * E - 1))
            nc.scalar.mul(c1sb[:, :], s1_ps[:, :], 1.0 / O)
            nc.sync.dma_start(out=ar1_in[:], in_=c1sb[:, :])
            nc.sync.dma_start(out=wdt[:, :], in_=wdt_d)
            # warm the exp activation table set (during the AR1 wait)
            nc.scalar.activation(tblw[:, :], ident[:, 0:16], AF.Exp)
            nc.leave_named_scope("s1", _sid, False)

            _sid, _ = nc.enter_named_scope("ar1", False)
            nc.gpsimd.collective_compute(
                "AllReduce", ALU.add, replica_groups=rg,
                ins=[ar1_in[:]], outs=[ar1_out[:]],
            )
            nc.sync.dma_start(out=s1full[:, :], in_=ar1_out[:])
            nc.leave_named_scope("ar1", _sid, False)

            # ---- squash1 on [b, (o,d)] then transpose to vT1 ----
            _sid, _ = nc.enter_named_scope("sq1", False)
            sq1 = wpool.tile([B, O * D], BF16, tag="sq1")
            nc.gpsimd.tensor_tensor(sq1[:, :], s1full[:, :], s1full[:, :],
                                    op=ALU.mult)
            sq1v = sq1[:, :].rearrange("p (o d) -> p o d", o=O)
            q1 = wpool.tile([B, O * 8], BF16, tag="q1")
            q1v = q1[:, :].rearrange("p (o d) -> p o d", o=O)
            nc.vector.tensor_tensor(q1v, sq1v[:, :, 0:8], sq1v[:, :, 8:16],
                                    op=ALU.add)
            q2 = wpool.tile([B, O * 4], BF16, tag="q2")
            q2v = q2[:, :].rearrange("p (o d) -> p o d", o=O)
            nc.vector.tensor_tensor(q2v, q1v[:, :, 0:4], q1v[:, :, 4:8],
                                    op=ALU.add)
            q3 = wpool.tile([B, O * 2], BF16, tag="q3")
            q3v = q3[:, :].rearrange("p (o d) -> p o d", o=O)
            nc.vector.tensor_tensor(q3v, q2v[:, :, 0:2], q2v[:, :, 2:4],
                                    op=ALU.add)
            n2s = wpool.tile([B, O], F32, tag="n2s")
            n2sv = n2s[:, :].rearrange("p (o d) -> p o d", o=O)
            nc.vector.tensor_tensor(n2sv, q3v[:, :, 0:1], q3v[:, :, 1:2],
                                    op=ALU.add)
            scl1 = wpool.tile([B, O], BF16, tag="scl1")
            emit_scale(n2s[:, :], scl1[:, :], B, O, "s1")
            nc.vector.memset(v1pad[:, :], 0.0)
            v1v = v1pad[:, :].rearrange("p (o dd) -> p o dd", o=O)[:, :, 0:D]
            s1fv = s1full[:, :].rearrange("p (o d) -> p o d", o=O)
            scl1b = scl1[:, :].unsqueeze(2).broadcast_to((B, O, D))
            nc.vector.tensor_tensor(v1v, s1fv, scl1b, op=ALU.mult)
            for h in range(4):
                tp = psq.tile([128, B], BF16, tag="rep")
                nc.tensor.transpose(tp[:, :], v1pad[:, h * 128:(h + 1) * 128],
                                    ident[:, :])
                nc.scalar.copy(vT1[:, h * B:(h + 1) * B], tp[:, :])
            nc.leave_named_scope("sq1", _sid, False)

            # pre-create the per-half s psum tiles (heartbeats slice them)
            psA2 = pss.tile([128, 2 * B + 64], F32, tag="sA")
            psB2 = pss.tile([128, 2 * B + 64], F32, tag="sB")
            nc.vector.memset(psA2[:, :], 0.0)
            nc.vector.memset(psB2[:, :], 0.0)

            def emit_ul(vT, dst_buf, scope, hb_ps, o_range=None, mid=None):
                _s, _ = nc.enter_named_scope(scope, False)
                for o in (o_range if o_range is not None else range(O)):
                    if mid is not None and o in mid:
                        mid[o]()
                    h, g = o // 4, o % 4
                    xu = wpool.tile([IL, IH * E * B], BF16, tag="xu")
                    usb = wpool.tile([IL, IH * E * B], BF16, tag="usb",
                                     bufs=3)
                    for ih in range(IH):
                        ups = psu.tile([IL, E * B], F32, tag="u")
                        for e in range(E):
                            off = (((h * IH + ih) * E) + e) * IL
                            lhsT = wdt[32 * g:32 * (g + 1), off:off + IL]
                            rhs = vT[32 * g:32 * (g + 1), h * B:(h + 1) * B]
                            nc.tensor.matmul(
                                ups[:, e * B:(e + 1) * B], lhsT, rhs,
                                start=True, stop=True,
                                tile_position=(32 * g, 0),
                            )
                        nc.scalar.copy(usb[:, ih * E * B:(ih + 1) * E * B],
                                       ups[:, :])
                    nc.vector.tensor_tensor(xu[:, :], xbf[:, :], usb[:, :],
                                            op=ALU.mult)
                    xuv = xu[:, :].rearrange("p (ih e2 r) -> p ih e2 r",
                                             ih=IH, e2=2)
                    r1 = wpool.tile([IL, IH * 4 * B], BF16, tag="r1")
                    r1o = r1[:, :].rearrange("p (ih r) -> p ih r", ih=IH)
                    nc.vector.tensor_tensor(r1o, xuv[:, :, 0], xuv[:, :, 1],
                                            op=ALU.add)
                    r1v = r1[:, :].rearrange("p (ih e2 r) -> p ih e2 r",
                                             ih=IH, e2=2)
                    r2 = wpool.tile([IL, IH * 2 * B], BF16, tag="r2")
                    r2o = r2[:, :].rearrange("p (ih r) -> p ih r", ih=IH)
                    nc.vector.tensor_tensor(r2o, r1v[:, :, 0], r1v[:, :, 1],
                                            op=ALU.add)
                    r2v = r2[:, :].rearrange("p (ih e2 r) -> p ih e2 r",
                                             ih=IH, e2=2)
                    dst = dst_buf[:, :].rearrange(
                        "p (o ih b) -> p o ih b", o=O, ih=IH)[:, o]
                    nc.gpsimd.tensor_tensor(dst, r2v[:, :, 0], r2v[:, :, 1],
                                            op=ALU.add)
                    hb(dst_buf[:, o * IH * B:o * IH * B + 64], hb_ps)
                nc.leave_named_scope(scope, _s, False)

            HALF = 8 * IH * B
            za3_t = {}

            def emit_exp_A():
                nc.scalar.activation(exp_buf[:, 0:HALF], l_buf[:, 0:HALF],
                                     AF.Exp)

            def emit_za_tree():
                za1 = wpool.tile([IL, 4 * IH * B], BF16, tag="za1")
                nc.vector.tensor_add(za1[:, :], exp_buf[:, 0:HALF // 2],
                                     exp_buf[:, HALF // 2:HALF])
                za2 = wpool.tile([IL, 2 * IH * B], BF16, tag="za2")
                nc.vector.tensor_add(za2[:, :], za1[:, 0:2 * IH * B],
                                     za1[:, 2 * IH * B:4 * IH * B])
                za3 = wpool.tile([IL, IH * B], BF16, tag="za3")
                nc.vector.tensor_add(za3[:, :], za2[:, 0:IH * B],
                                     za2[:, IH * B:2 * IH * B])
                za3_t["za3"] = za3

            def emit_softmax_B(scope):
                """exp of l_buf half B, zb tree, final Z, rz, xp."""
                _s, _ = nc.enter_named_scope(scope, False)
                nc.scalar.activation(exp_buf[:, HALF:2 * HALF],
                                     l_buf[:, HALF:2 * HALF], AF.Exp)
                zb1 = wpool.tile([IL, 4 * IH * B], BF16, tag="zb1")
                nc.vector.tensor_add(zb1[:, :], exp_buf[:, HALF:HALF + HALF // 2],
                                     exp_buf[:, HALF + HALF // 2:2 * HALF])
                za3_t["zb1"] = zb1
                zb2 = wpool.tile([IL, 2 * IH * B], BF16, tag="zb2")
                nc.vector.tensor_add(zb2[:, :], zb1[:, 0:2 * IH * B],
                                     zb1[:, 2 * IH * B:4 * IH * B])
                zb3 = wpool.tile([IL, IH * B], BF16, tag="zb3")
                nc.vector.tensor_add(zb3[:, :], zb2[:, 0:IH * B],
                                     zb2[:, IH * B:2 * IH * B])
                zbuf = wpool.tile([IL, IH * B], BF16, tag="z")
                nc.vector.tensor_add(zbuf[:, :], za3_t["za3"][:, :], zb3[:, :])
                za3_t["z"] = zbuf
                rz = wpool.tile([IL, IH * B], F32, tag="rz")
                nc.vector.reciprocal(rz[:, :], zbuf[:, :])
                rzbf = wpool.tile([IL, IH * B], BF16, tag="rzbf")
                nc.vector.tensor_copy(rzbf[:, :], rz[:, :])
                nc.vector.tensor_tensor(
                    xp[:, :].rearrange("p (ih e b) -> p ih e b", ih=IH, e=E),
                    xbf[:, :].rearrange("p (ih e b) -> p ih e b", ih=IH, e=E),
                    rzbf[:, :].rearrange("p (ih b) -> p ih b", ih=IH)
                    .unsqueeze(2).broadcast_to((IL, IH, E, B)),
                    op=ALU.mult,
                )
                hb(xp[:, 0:64], psA2)
                nc.leave_named_scope(scope, _s, False)

            def emit_squash_half(sTp, vT_dst_half, tag):
                """squash on transposed [(g,dd),(hh,b)] half -> vT half."""
                sq = wpool.tile([128, 2 * B], BF16, tag="sqh", bufs=1)
                nc.gpsimd.tensor_tensor(sq[:, :], sTp[:, :], sTp[:, :],
                                        op=ALU.mult)
                n2p = psq.tile([4, 2 * B], F32, tag="n2")
                nc.tensor.matmul(n2p[:, :], sel4[:, :], sq[:, :],
                                 start=True, stop=True)
                n2h = wpool.tile([4, 2 * B], F32, tag="n2h", bufs=1)
                nc.vector.tensor_copy(n2h[:, :], n2p[:, :])
                sclh = wpool.tile([4, 2 * B], BF16, tag="sclh", bufs=1)
                emit_scale(n2h[:, :], sclh[:, :], 4, 2 * B, tag)
                rep = psq.tile([128, 2 * B], F32, tag="rep")
                nc.tensor.matmul(rep[:, :], selT[:, :], sclh[:, :],
                                 start=True, stop=True)
                nc.vector.tensor_tensor(vT_dst_half, sTp[:, :], rep[:, :],
                                        op=ALU.mult)

            def warm_pe():
                """~4us of dense matmuls to push HAM to K=8/8 before the
                s-matmul phase; gated on the softmax Z so it runs during the
                rz/xp tail instead of racing ahead."""
                wb = psq.tile([4, 2 * B], F32, tag="n2")
                zt = za3_t["zb1"]
                nc.tensor.matmul(wb[:, :], sel4[:, :], zt[:, 0:2 * B],
                                 start=True, stop=True)
                for k in range(20):
                    nc.tensor.matmul(wb[:, :], sel4[:, :], xbf[:, 0:2 * B],
                                     start=True, stop=True)

            def emit_xcs(itr, psA, psB):
                """softmax-weighted xc + col-tiled s matmuls; itr==2 fires the
                two half-ARs (squash emitted later, by the caller); itr==3
                DMAs the partial s out."""
                scope = f"xcs{itr}"
                _s, _ = nc.enter_named_scope(scope, False)
                for o in range(O):
                    h, g = o // 4, o % 4
                    ps, hh = (psA, h) if o < 8 else (psB, h - 2)
                    xc = wpool.tile([IL, IH * E * B], BF16, tag="xc", bufs=3)
                    nc.vector.tensor_tensor(
                        xc[:, :].rearrange("p (ih e b) -> p ih e b",
                                           ih=IH, e=E),
                        exp_buf[:, :].rearrange("p (o ih b) -> p o ih b",
                                                o=O, ih=IH)[:, o]
                        .unsqueeze(2).broadcast_to((IL, IH, E, B)),
                        xp[:, :].rearrange("p (ih e b) -> p ih e b",
                                           ih=IH, e=E),
                        op=ALU.mult,
                    )
                    for t in range(IH * E):
                        nc.tensor.matmul(
                            ps[32 * g:32 * g + D, hh * B:(hh + 1) * B],
                            wst_v[:, t, o], xc[:, t * B:(t + 1) * B],
                            start=(t == 0), stop=(t == IH * E - 1),
                            tile_position=(0, 32 * g),
                        )
                    if o == 7:
                        if itr == 2:
                            _s2, _ = nc.enter_named_scope("ar2a", False)
                            nc.scalar.copy(c2a[:, :], psA[:, 0:2 * B])
                            nc.sync.dma_start(out=ar2a_in[:], in_=c2a[:, :])
                            nc.gpsimd.collective_compute(
                                "AllReduce", ALU.add, replica_groups=rg,
                                ins=[ar2a_in[:]], outs=[ar2a_out[:]],
                            )
                            nc.sync.dma_start(out=sTa[:, :], in_=ar2a_out[:])
                            nc.leave_named_scope("ar2a", _s2, False)
                        else:
                            nc.scalar.copy(sp_sb[:, 0:2 * B], psA[:, 0:2 * B])
                if itr == 2:
                    _s2, _ = nc.enter_named_scope("ar2b", False)
                    nc.scalar.copy(c2b[:, :], psB[:, 0:2 * B])
                    nc.sync.dma_start(out=ar2b_in[:], in_=c2b[:, :])
                    nc.gpsimd.collective_compute(
                        "AllReduce", ALU.add, replica_groups=rg,
                        ins=[ar2b_in[:]], outs=[ar2b_out[:]],
                    )
                    nc.sync.dma_start(out=sTb[:, :], in_=ar2b_out[:])
                    nc.leave_named_scope("ar2b", _s2, False)
                else:
                    nc.scalar.copy(sp_sb[:, 2 * B:4 * B], psB[:, 0:2 * B])
                    nc.sync.dma_start(out=sp_out, in_=sp_sb[:, :])
                nc.leave_named_scope(scope, _s, False)

            # ---- iteration 2 ----
            emit_ul(vT1, l_buf, "ul2", psA2,
                    mid={8: emit_exp_A, 10: emit_za_tree})
            emit_softmax_B("sm2")
            warm_pe()
            emit_xcs(2, psA2, psB2)

            # ---- iteration 3 (half-pipelined against the two iter-2 ARs) ---
            _sid, _ = nc.enter_named_scope("sq2a", False)
            emit_squash_half(sTa, vT2[:, 0:2 * B], "a")
            nc.leave_named_scope("sq2a", _sid, False)
            emit_ul(vT2, delta_buf, "ul3a", psA2, o_range=range(8))
            _sid, _ = nc.enter_named_scope("dl3a", False)
            nc.vector.tensor_add(l_buf[:, 0:HALF], l_buf[:, 0:HALF],
                                 delta_buf[:, 0:HALF])
            emit_exp_A()
            emit_za_tree()
            nc.leave_named_scope("dl3a", _sid, False)
            _sid, _ = nc.enter_named_scope("sq2b", False)
            emit_squash_half(sTb, vT2[:, 2 * B:4 * B], "b")
            nc.leave_named_scope("sq2b", _sid, False)
            emit_ul(vT2, delta_buf, "ul3b", psA2, o_range=range(8, 16))
            _sid, _ = nc.enter_named_scope("dl3b", False)
            nc.vector.tensor_add(l_buf[:, HALF:2 * HALF],
                                 l_buf[:, HALF:2 * HALF],
                                 delta_buf[:, HALF:2 * HALF])
            nc.leave_named_scope("dl3b", _sid, False)
            emit_softmax_B("sm3")
            warm_pe()
            emit_xcs(3, psA2, psB2)

    nc.compile()
    return nc


def _host_prep(x, weight):
    """Build the per-core input maps (free host-side rearrangement)."""
    in_maps = []
    sel4 = np.zeros((128, 4), dtype=np.float32)
    selT = np.zeros((4, 128), dtype=np.float32)
    for g in range(4):
        sel4[32 * g:32 * (g + 1), g] = 1.0
        selT[g, 32 * g:32 * (g + 1)] = 1.0
    ident = np.eye(128, dtype=np.float32)
    for c in range(N_CORES):
        x_c = x[:, c * IC:(c + 1) * IC, :]          # [B, 256, E]
        w_c = weight[:, c * IC:(c + 1) * IC, :, :]  # [O, 256, D, E]

        # xbf [il, (ih, e, b)]
        xr = x_c.reshape(B, IH, IL, E)              # b, ih, il, e
        xt = np.ascontiguousarray(
            xr.transpose(2, 1, 3, 0)                # il, ih, e, b
        ).reshape(IL, IH * E * B)

        wr = w_c.reshape(4, 4, IH, IL, D, E)        # h, g, ih, il, d, e

        # wdt [(g, dd=32), (h, ih, e, il)] (dd >= 16 zero)
        wdtv = np.zeros((4, 32, 4, IH, E, IL), dtype=np.float32)
        wdtv[:, :D] = wr.transpose(1, 4, 0, 2, 5, 3)  # g, d, h, ih, e, il
        wdt = wdtv.reshape(128, 4 * IH * E * IL)

        # wst [il, (ih, e, o, d)]
        wst = np.ascontiguousarray(
            w_c.reshape(O, IH, IL, D, E).transpose(2, 1, 4, 0, 3)
        ).reshape(IL, IH * E * O * D)

        in_maps.append({
            "xbf": xt.astype(NPBF16),
            "wdt": wdt.astype(NPBF16),
            "wst": wst.astype(NPBF16),
            "sel4": sel4.astype(NPBF16),
            "selT": selT.astype(NPBF16),
            "ident": ident.astype(NPBF16),
        })
    return in_maps


def _host_finish(partials):
    """Sum the 8 per-core partial s3 tensors [(g,dd32),(h,b)], final squash."""
    EPS = 1e-8
    acc = np.zeros((128, 4 * B), dtype=np.float64)
    for p in partials:
        acc += p.astype(np.float64)
    sp = acc.reshape(4, DD, 4, B)                   # g, dd, h, b
    s = np.zeros((B, O, D), dtype=np.float64)
    for o in range(O):
        h, g = o // 4, o % 4
        s[:, o, :] = sp[g, 0:D, h, :].T             # [B, D]
    n2 = (s * s).sum(axis=-1, keepdims=True)
    n = np.sqrt(n2)
    v = (n2 / (1.0 + n2) / (n + EPS)) * s
    return v.astype(np.float32)


def kernel(x, weight, _trace=False):
    x = np.asarray(x, dtype=np.float32)
    weight = np.asarray(weight, dtype=np.float32)
    if "nc" not in _CACHE:
        _CACHE["nc"] = build()
    nc = _CACHE["nc"]
    in_maps = _host_prep(x, weight)
    res = run_bass_kernel_spmd(
        nc, in_maps, core_ids=list(range(N_CORES)), trace=_trace
    )
    out = _host_finish([res.results[c]["sp"] for c in range(N_CORES)])
    if _trace:
        _CACHE["last_result"] = res
    return out


if __name__ == "__main__":
    rng = np.random.default_rng(0)
    x = rng.standard_normal((B, I_FULL, E)).astype(np.float32)
    w = (0.01 * rng.standard_normal((O, I_FULL, D, E))).astype(np.float32)
    out = kernel(x, w)
    print("out", out.shape, out.dtype, np.abs(out).max())

